# revision 14
# baseline (speedup 1.0000x reference)
"""ECT layer (segment_reduce) Trainium2 kernel.

Math (matches the jax reference):
    nh  = x @ v                          [N, T]
    ecc = sigmoid(SCALE*(lin_r - nh))    [R, N, T]
    ect = segment_sum(ecc over N by index) -> [B, R, T]
    out = ect / max(ect over (R,T) per b)

Sharding: data-parallel over point clouds (bins). Core c owns global bins
[4c, 4c+4); the host routes every point to its bin's core, so no cross-core
reduction is needed. The host also precomputes the cheap [N,3]x[3,32]
projection nh (9.6 MFLOP) and the per-tile one-hot matrices; the device does
the heavy part (102M sigmoids + 102M-MAC segment reduction). Per core,
points are processed in 104 tiles of 128 (partition dim = points), fused in
pairs:
    DVE/GPSIMD : z = linb - nh  (nh broadcast over the R axis)  [128, 2048]
    ACT        : ecc = sigmoid(z)  (written as fp32r)           [128, 2048]
    PE         : ect += onehot.T @ ecc  (fp32r, PSUM accum)     [4, 1024]
Epilogue: per-bin max over R*T, multiply by reciprocal, DMA out.
"""

import numpy as np

N = 100000
B = 32
R = 32
T = 32
D = 3
SCALE = 100.0

NCORES = 8
BLOC = B // NCORES        # local bins per core
CAP = 13312               # per-core point capacity (104 tiles of 128)
PTILE = 128
TILES = CAP // PTILE      # 104
GROUPS = TILES // 2       # tile pairs
F = R * T                 # 1024 output features per bin
FH = F // 2               # 512, max moving free dim per matmul
RSPLIT = 20               # r < RSPLIT handled by DVE, rest by GPSIMD

_cache = {}


def _build():
    """Build + bacc-compile the SPMD program once per process."""
    import concourse.tile as tile
    from concourse import bacc, mybir

    nc = bacc.Bacc("TRN2", target_bir_lowering=False, debug=False,
                   num_devices=NCORES)
    f32 = mybir.dt.float32
    f32r = mybir.dt.float32r

    nh_d = nc.dram_tensor("nhT", [PTILE, TILES * T], f32,
                          kind="ExternalInput")
    linb_d = nc.dram_tensor("linb2", [PTILE, 2 * F], f32,
                            kind="ExternalInput")
    oh_d = nc.dram_tensor("ohT", [PTILE, TILES * BLOC], f32,
                          kind="ExternalInput")
    out_d = nc.dram_tensor("out", [BLOC, F], f32, kind="ExternalOutput")

    # fp32r accuracy probes (run once, independent of the main pipeline)
    pc_d = nc.dram_tensor("pc", [PTILE, BLOC], f32, kind="ExternalInput")
    pd_d = nc.dram_tensor("pd", [PTILE, FH], f32, kind="ExternalInput")
    pseg_d = nc.dram_tensor("pseg", [BLOC, FH], f32, kind="ExternalOutput")

    NH_CHUNKS = 8
    CW = (TILES * T) // NH_CHUNKS

    with tile.TileContext(nc) as tc:
        with (
            tc.tile_pool(name="singles", bufs=1) as singles,
            tc.tile_pool(name="work", bufs=3) as work,
            tc.tile_pool(name="post", bufs=1) as post,
            tc.tile_pool(name="psacc", bufs=1, space="PSUM") as psacc,
            tc.tile_pool(name="psprobe", bufs=1, space="PSUM") as psprobe,
        ):
            NH = singles.tile([PTILE, TILES * T], f32)
            LINB = singles.tile([PTILE, 2 * F], f32)
            OHF = singles.tile([PTILE, TILES * BLOC], f32)
            for ch in range(NH_CHUNKS):
                nc.sync.dma_start(out=NH[:, ch * CW:(ch + 1) * CW],
                                  in_=nh_d.ap()[:, ch * CW:(ch + 1) * CW])
            nc.sync.dma_start(out=LINB, in_=linb_d.ap())
            nc.sync.dma_start(out=OHF, in_=oh_d.ap())
            OHR = singles.tile([PTILE, TILES * BLOC], f32r)
            nc.vector.tensor_copy(out=OHR, in_=OHF)

            ect = psacc.tile([BLOC, F], f32)

            linb4 = LINB.rearrange("p (j r t) -> p j r t", j=2, t=T)
            for g in range(GROUPS):
                # z[p, (j, r, t)] = linb[p, r, t] - nh[p, tile j, t]
                z = work.tile([PTILE, 2 * F], f32)
                z4 = z.rearrange("p (j r t) -> p j r t", j=2, t=T)
                nh4 = NH[:, (2 * g) * T:(2 * g + 2) * T] \
                    .rearrange("p (j r t) -> p j r t", j=2, r=1) \
                    .broadcast_to([PTILE, 2, R, T])
                nc.vector.tensor_tensor(
                    out=z4[:, :, :RSPLIT, :],
                    in0=linb4[:, :, :RSPLIT, :],
                    in1=nh4[:, :, :RSPLIT, :],
                    op=mybir.AluOpType.subtract,
                )
                nc.gpsimd.tensor_tensor(
                    out=z4[:, :, RSPLIT:, :],
                    in0=linb4[:, :, RSPLIT:, :],
                    in1=nh4[:, :, RSPLIT:, :],
                    op=mybir.AluOpType.subtract,
                )

                ecc = work.tile([PTILE, 2 * F], f32r)
                nc.scalar.activation(
                    out=ecc, in_=z,
                    func=mybir.ActivationFunctionType.Sigmoid,
                )

                for j in range(2):
                    i = 2 * g + j
                    for h in range(2):
                        nc.tensor.matmul(
                            out=ect[:, h * FH:(h + 1) * FH],
                            lhsT=OHR[:, i * BLOC:(i + 1) * BLOC],
                            rhs=ecc[:, j * F + h * FH:j * F + (h + 1) * FH],
                            start=(i == 0), stop=(i == TILES - 1),
                        )

            # normalize: out = ect * (1 / max(ect, axis=free))
            mx = post.tile([BLOC, 1], f32)
            nc.vector.tensor_reduce(
                out=mx, in_=ect,
                axis=mybir.AxisListType.X, op=mybir.AluOpType.max,
            )
            rmx = post.tile([BLOC, 1], f32)
            nc.vector.reciprocal(out=rmx, in_=mx)
            outn = post.tile([BLOC, F], f32)
            nc.vector.tensor_scalar(
                out=outn, in0=ect,
                scalar1=rmx, scalar2=None,
                op0=mybir.AluOpType.mult,
            )
            nc.sync.dma_start(out=out_d.ap(), in_=outn)

            # ---- fp32r probe ----
            PC = post.tile([PTILE, BLOC], f32)
            PD = post.tile([PTILE, FH], f32)
            nc.sync.dma_start(out=PC, in_=pc_d.ap())
            nc.sync.dma_start(out=PD, in_=pd_d.ap())
            PCr = post.tile([PTILE, BLOC], f32r)
            PDr = post.tile([PTILE, FH], f32r)
            nc.vector.tensor_copy(out=PCr, in_=PC)
            nc.vector.tensor_copy(out=PDr, in_=PD)
            pseg_ps = psprobe.tile([BLOC, FH], f32)
            nc.tensor.matmul(out=pseg_ps, lhsT=PCr,
                             rhs=PDr, start=True, stop=True)
            pseg_sb = post.tile([BLOC, FH], f32)
            nc.vector.tensor_copy(out=pseg_sb, in_=pseg_ps)
            nc.sync.dma_start(out=pseg_d.ap(), in_=pseg_sb)

    nc.compile()
    return nc


def _host_prep(x, v, lin, index):
    """Route points to their bin's core; build per-core input maps."""
    x = np.asarray(x, dtype=np.float32)
    v = np.asarray(v, dtype=np.float32)
    lin100 = (SCALE * np.asarray(lin, dtype=np.float32)).reshape(R)
    linb_row = np.repeat(lin100, T)                      # [F], f = r*T + t
    linb2 = np.ascontiguousarray(np.broadcast_to(
        np.concatenate([linb_row, linb_row]), (PTILE, 2 * F)))

    order = np.argsort(index, kind="stable")
    counts = np.bincount(index, minlength=B)
    group_counts = counts.reshape(NCORES, BLOC).sum(axis=1)
    if group_counts.max() > CAP:
        return None  # fall back to host compute
    starts = np.concatenate([[0], np.cumsum(group_counts)[:-1]])

    nh100 = x @ (SCALE * v)                              # [N, T] f32

    # probe data (same for every core)
    rng = np.random.default_rng(0)
    pc = (rng.integers(0, BLOC, PTILE)[:, None]
          == np.arange(BLOC)[None, :]).astype(np.float32)
    pd = (1.0 / (1.0 + np.exp(-rng.standard_normal((PTILE, FH))))
          ).astype(np.float32)

    in_maps = []
    for c in range(NCORES):
        pts = order[starts[c]:starts[c] + group_counts[c]]
        n_c = len(pts)
        nh_c = np.zeros((CAP, T), dtype=np.float32)
        nh_c[:n_c] = nh100[pts]
        # nhT[p, i*T + t] = nh100 of point (i*PTILE + p)
        nhT = np.ascontiguousarray(
            nh_c.reshape(TILES, PTILE, T).transpose(1, 0, 2)
            .reshape(PTILE, TILES * T))
        idxf = np.full(CAP, -1, dtype=np.int64)
        idxf[:n_c] = index[pts] - c * BLOC
        # ohT[p, i*BLOC + b] = 1.0 iff point (i*PTILE + p) is in local bin b
        oh = (idxf.reshape(TILES, PTILE)[:, :, None]
              == np.arange(BLOC)[None, None, :]).astype(np.float32)
        ohT = np.ascontiguousarray(
            oh.transpose(1, 0, 2).reshape(PTILE, TILES * BLOC))
        in_maps.append({
            "nhT": nhT, "linb2": linb2, "ohT": ohT,
            "pc": pc, "pd": pd,
        })
    probes = {"pc": pc, "pd": pd}
    return in_maps, probes


def _host_fallback(x, v, lin, index):
    """Pure-numpy reference path (pathological index distributions only)."""
    x = np.asarray(x, dtype=np.float32)
    v = np.asarray(v, dtype=np.float32)
    lin = np.asarray(lin, dtype=np.float32).reshape(R, 1, 1)
    ect = np.zeros((B, R, T), dtype=np.float32)
    for s in range(0, len(x), 4096):
        xc = x[s:s + 4096]
        ic = index[s:s + 4096]
        nh = xc @ v                                   # [n, T]
        z = SCALE * (lin - nh[None, :, :])            # [R, n, T]
        ecc = 1.0 / (1.0 + np.exp(-z))
        np.add.at(ect, ic, np.transpose(ecc, (1, 0, 2)).astype(np.float32))
    return ect / ect.max(axis=(1, 2), keepdims=True)


def kernel(x, v, lin, index):
    from concourse import bass_utils

    x = np.asarray(x)
    v = np.asarray(v)
    lin = np.asarray(lin)
    index = np.asarray(index)

    prep = _host_prep(x, v, lin, index)
    if prep is None:
        return _host_fallback(x, v, lin, index)
    in_maps, _ = prep

    if "nc" not in _cache:
        _cache["nc"] = _build()
    nc = _cache["nc"]

    res = bass_utils.run_bass_kernel_spmd(nc, in_maps, list(range(NCORES)))
    out = np.concatenate(
        [res.results[c]["out"].reshape(BLOC, R, T) for c in range(NCORES)],
        axis=0,
    )
    return out.astype(np.float32)


# revision 16
# speedup vs baseline: 1.0161x; 1.0161x over previous
"""ECT layer (segment_reduce) Trainium2 kernel.

Math (matches the jax reference):
    nh  = x @ v                          [N, T]
    ecc = sigmoid(SCALE*(lin_r - nh))    [R, N, T]
    ect = segment_sum(ecc over N by index) -> [B, R, T]
    out = ect / max(ect over (R,T) per b)

Sharding: data-parallel over point clouds (bins). Core c owns global bins
[4c, 4c+4); the host routes every point to its bin's core, so no cross-core
reduction is needed. The host also precomputes the cheap [N,3]x[3,32]
projection nh (9.6 MFLOP) and the per-tile one-hot matrices; the device does
the heavy part (102M sigmoids + 102M-MAC segment reduction). Per core,
points are processed in 104 tiles of 128 (partition dim = points), fused in
groups of 4 tiles:
    DVE (3 tiles) + GPSIMD (1 tile): z = linb - nh (nh broadcast over R)
    ACT : ecc = sigmoid(z) over the whole group [128, 4096] (fp32r out)
    PE  : ect += onehot.T @ ecc  (fp32r, two alternating PSUM accumulators)
Epilogue: add the accumulators, per-bin max over R*T, multiply by
reciprocal, DMA out.
"""

import numpy as np

N = 100000
B = 32
R = 32
T = 32
D = 3
SCALE = 100.0

NCORES = 8
BLOC = B // NCORES        # local bins per core
CAP = 13312               # per-core point capacity (104 tiles of 128)
PTILE = 128
TILES = CAP // PTILE      # 104
GTILES = 4                # tiles per fused group
GROUPS = TILES // GTILES  # 26
F = R * T                 # 1024 output features per bin
FH = F // 2               # 512, max moving free dim per matmul

_cache = {}


def _build():
    """Build + bacc-compile the SPMD program once per process."""
    import concourse.tile as tile
    from concourse import bacc, mybir

    nc = bacc.Bacc("TRN2", target_bir_lowering=False, debug=False,
                   num_devices=NCORES)
    f32 = mybir.dt.float32
    f32r = mybir.dt.float32r

    nh_d = nc.dram_tensor("nhT", [PTILE, TILES * T], f32,
                          kind="ExternalInput")
    linb_d = nc.dram_tensor("linb", [PTILE, F], f32, kind="ExternalInput")
    oh_d = nc.dram_tensor("ohT", [PTILE, TILES * BLOC], f32,
                          kind="ExternalInput")
    out_d = nc.dram_tensor("out", [BLOC, F], f32, kind="ExternalOutput")

    # fp32r accuracy probe (runs once, independent of the main pipeline)
    pc_d = nc.dram_tensor("pc", [PTILE, BLOC], f32, kind="ExternalInput")
    pd_d = nc.dram_tensor("pd", [PTILE, FH], f32, kind="ExternalInput")
    pseg_d = nc.dram_tensor("pseg", [BLOC, FH], f32, kind="ExternalOutput")

    NH_CHUNKS = 8
    CW = (TILES * T) // NH_CHUNKS

    with tile.TileContext(nc) as tc:
        with (
            tc.tile_pool(name="singles", bufs=1) as singles,
            tc.tile_pool(name="work", bufs=2) as work,
            tc.tile_pool(name="post", bufs=1) as post,
            tc.tile_pool(name="psacc", bufs=1, space="PSUM") as psacc,
            tc.tile_pool(name="psprobe", bufs=1, space="PSUM") as psprobe,
        ):
            NH = singles.tile([PTILE, TILES * T], f32)
            LINB = singles.tile([PTILE, F], f32)
            OHF = singles.tile([PTILE, TILES * BLOC], f32)
            for ch in range(NH_CHUNKS):
                nc.sync.dma_start(out=NH[:, ch * CW:(ch + 1) * CW],
                                  in_=nh_d.ap()[:, ch * CW:(ch + 1) * CW])
            nc.sync.dma_start(out=LINB, in_=linb_d.ap())
            nc.sync.dma_start(out=OHF, in_=oh_d.ap())
            OHR = singles.tile([PTILE, TILES * BLOC], f32r)
            nc.vector.tensor_copy(out=OHR, in_=OHF)

            linb3 = LINB.rearrange("p (r t) -> p r t", t=T)
            ect0 = psacc.tile([BLOC, F], f32, tag="ect0")
            ect1 = psacc.tile([BLOC, F], f32, tag="ect1")
            accs = (ect0, ect1)

            for g in range(GROUPS):
                z = work.tile([PTILE, GTILES * F], f32)
                for j in range(GTILES):
                    i = GTILES * g + j
                    z3 = z[:, j * F:(j + 1) * F] \
                        .rearrange("p (r t) -> p r t", t=T)
                    nh3 = NH[:, i * T:(i + 1) * T] \
                        .rearrange("p (r t) -> p r t", r=1) \
                        .broadcast_to([PTILE, R, T])
                    eng = nc.gpsimd if j == GTILES - 1 else nc.vector
                    eng.tensor_tensor(
                        out=z3, in0=linb3, in1=nh3,
                        op=mybir.AluOpType.subtract,
                    )

                ecc = work.tile([PTILE, GTILES * F], f32r)
                nc.scalar.activation(
                    out=ecc, in_=z,
                    func=mybir.ActivationFunctionType.Sigmoid,
                )

                for j in range(GTILES):
                    i = GTILES * g + j
                    acc = accs[i % 2]
                    for h in range(2):
                        nc.tensor.matmul(
                            out=acc[:, h * FH:(h + 1) * FH],
                            lhsT=OHR[:, i * BLOC:(i + 1) * BLOC],
                            rhs=ecc[:, j * F + h * FH:j * F + (h + 1) * FH],
                            start=(i < 2), stop=(i >= TILES - 2),
                        )

            # normalize: out = ect * (1 / max(ect, axis=free))
            ect1s = post.tile([BLOC, F], f32)
            nc.vector.tensor_copy(out=ect1s, in_=ect1)
            ectsum = post.tile([BLOC, F], f32)
            nc.vector.tensor_tensor(out=ectsum, in0=ect0, in1=ect1s,
                                    op=mybir.AluOpType.add)
            mx = post.tile([BLOC, 1], f32)
            nc.vector.tensor_reduce(
                out=mx, in_=ectsum,
                axis=mybir.AxisListType.X, op=mybir.AluOpType.max,
            )
            rmx = post.tile([BLOC, 1], f32)
            nc.vector.reciprocal(out=rmx, in_=mx)
            outn = post.tile([BLOC, F], f32)
            nc.vector.tensor_scalar(
                out=outn, in0=ectsum,
                scalar1=rmx, scalar2=None,
                op0=mybir.AluOpType.mult,
            )
            nc.sync.dma_start(out=out_d.ap(), in_=outn)

            # ---- fp32r probe ----
            PC = post.tile([PTILE, BLOC], f32)
            PD = post.tile([PTILE, FH], f32)
            nc.sync.dma_start(out=PC, in_=pc_d.ap())
            nc.sync.dma_start(out=PD, in_=pd_d.ap())
            PCr = post.tile([PTILE, BLOC], f32r)
            PDr = post.tile([PTILE, FH], f32r)
            nc.vector.tensor_copy(out=PCr, in_=PC)
            nc.vector.tensor_copy(out=PDr, in_=PD)
            pseg_ps = psprobe.tile([BLOC, FH], f32)
            nc.tensor.matmul(out=pseg_ps, lhsT=PCr,
                             rhs=PDr, start=True, stop=True)
            pseg_sb = post.tile([BLOC, FH], f32)
            nc.vector.tensor_copy(out=pseg_sb, in_=pseg_ps)
            nc.sync.dma_start(out=pseg_d.ap(), in_=pseg_sb)

    nc.compile()
    return nc


def _host_prep(x, v, lin, index):
    """Route points to their bin's core; build per-core input maps."""
    x = np.asarray(x, dtype=np.float32)
    v = np.asarray(v, dtype=np.float32)
    lin100 = (SCALE * np.asarray(lin, dtype=np.float32)).reshape(R)
    linb_row = np.repeat(lin100, T)                      # [F], f = r*T + t
    linb = np.ascontiguousarray(np.broadcast_to(linb_row, (PTILE, F)))

    order = np.argsort(index, kind="stable")
    counts = np.bincount(index, minlength=B)
    group_counts = counts.reshape(NCORES, BLOC).sum(axis=1)
    if group_counts.max() > CAP:
        return None  # fall back to host compute
    starts = np.concatenate([[0], np.cumsum(group_counts)[:-1]])

    nh100 = x @ (SCALE * v)                              # [N, T] f32

    # probe data (same for every core)
    rng = np.random.default_rng(0)
    pc = (rng.integers(0, BLOC, PTILE)[:, None]
          == np.arange(BLOC)[None, :]).astype(np.float32)
    pd = (1.0 / (1.0 + np.exp(-rng.standard_normal((PTILE, FH))))
          ).astype(np.float32)

    in_maps = []
    for c in range(NCORES):
        pts = order[starts[c]:starts[c] + group_counts[c]]
        n_c = len(pts)
        nh_c = np.zeros((CAP, T), dtype=np.float32)
        nh_c[:n_c] = nh100[pts]
        # nhT[p, i*T + t] = nh100 of point (i*PTILE + p)
        nhT = np.ascontiguousarray(
            nh_c.reshape(TILES, PTILE, T).transpose(1, 0, 2)
            .reshape(PTILE, TILES * T))
        idxf = np.full(CAP, -1, dtype=np.int64)
        idxf[:n_c] = index[pts] - c * BLOC
        # ohT[p, i*BLOC + b] = 1.0 iff point (i*PTILE + p) is in local bin b
        oh = (idxf.reshape(TILES, PTILE)[:, :, None]
              == np.arange(BLOC)[None, None, :]).astype(np.float32)
        ohT = np.ascontiguousarray(
            oh.transpose(1, 0, 2).reshape(PTILE, TILES * BLOC))
        in_maps.append({
            "nhT": nhT, "linb": linb, "ohT": ohT,
            "pc": pc, "pd": pd,
        })
    probes = {"pc": pc, "pd": pd}
    return in_maps, probes


def _host_fallback(x, v, lin, index):
    """Pure-numpy reference path (pathological index distributions only)."""
    x = np.asarray(x, dtype=np.float32)
    v = np.asarray(v, dtype=np.float32)
    lin = np.asarray(lin, dtype=np.float32).reshape(R, 1, 1)
    ect = np.zeros((B, R, T), dtype=np.float32)
    for s in range(0, len(x), 4096):
        xc = x[s:s + 4096]
        ic = index[s:s + 4096]
        nh = xc @ v                                   # [n, T]
        z = SCALE * (lin - nh[None, :, :])            # [R, n, T]
        ecc = 1.0 / (1.0 + np.exp(-z))
        np.add.at(ect, ic, np.transpose(ecc, (1, 0, 2)).astype(np.float32))
    return ect / ect.max(axis=(1, 2), keepdims=True)


def kernel(x, v, lin, index):
    from concourse import bass_utils

    x = np.asarray(x)
    v = np.asarray(v)
    lin = np.asarray(lin)
    index = np.asarray(index)

    prep = _host_prep(x, v, lin, index)
    if prep is None:
        return _host_fallback(x, v, lin, index)
    in_maps, _ = prep

    if "nc" not in _cache:
        _cache["nc"] = _build()
    nc = _cache["nc"]

    res = bass_utils.run_bass_kernel_spmd(nc, in_maps, list(range(NCORES)))
    out = np.concatenate(
        [res.results[c]["out"].reshape(BLOC, R, T) for c in range(NCORES)],
        axis=0,
    )
    return out.astype(np.float32)


# revision 18
# speedup vs baseline: 1.2599x; 1.2400x over previous
"""ECT layer (segment_reduce) Trainium2 kernel.

Math (matches the jax reference):
    nh  = x @ v                          [N, T]
    ecc = sigmoid(SCALE*(lin_r - nh))    [R, N, T]
    ect = segment_sum(ecc over N by index) -> [B, R, T]
    out = ect / max(ect over (R,T) per b)

Sharding: data-parallel over point clouds (bins). Core c owns global bins
[4c, 4c+4); the host routes every point to its bin's core, so no cross-core
reduction is needed. The host also precomputes the cheap [N,3]x[3,32]
projection nh (9.6 MFLOP) and the per-tile one-hot matrices; the device does
the heavy part (102M sigmoids + 102M-MAC segment reduction). Per core,
points are processed in 104 tiles of 128 (partition dim = points), fused in
groups of 4 tiles:
    DVE (3 tiles) + GPSIMD (1 tile): z = linb - nh (nh broadcast over R)
    ACT : ecc = sigmoid(z) over the whole group [128, 4096] (fp32r out)
    PE  : ect += onehot.T @ ecc  (fp32r, two alternating PSUM accumulators)
Epilogue: add the accumulators, per-bin max over R*T, multiply by
reciprocal, DMA out.
"""

import numpy as np

N = 100000
B = 32
R = 32
T = 32
D = 3
SCALE = 100.0

NCORES = 8
BLOC = B // NCORES        # local bins per core
CAP = 13312               # per-core point capacity (104 tiles of 128)
PTILE = 128
TILES = CAP // PTILE      # 104
GTILES = 4                # tiles per fused group
GROUPS = TILES // GTILES  # 26
F = R * T                 # 1024 output features per bin
FH = F // 2               # 512, max moving free dim per matmul

_cache = {}


def _build():
    """Build + bacc-compile the SPMD program once per process."""
    import concourse.tile as tile
    from concourse import bacc, mybir

    nc = bacc.Bacc("TRN2", target_bir_lowering=False, debug=False,
                   num_devices=NCORES)
    f32 = mybir.dt.float32
    f32r = mybir.dt.float32r

    nh_d = nc.dram_tensor("nhT", [PTILE, TILES * T], f32,
                          kind="ExternalInput")
    linb_d = nc.dram_tensor("linb", [PTILE, F], f32, kind="ExternalInput")
    oh_d = nc.dram_tensor("ohT", [PTILE, TILES * BLOC], f32,
                          kind="ExternalInput")
    out_d = nc.dram_tensor("out", [BLOC, F], f32, kind="ExternalOutput")

    # fp32r accuracy probe (runs once, independent of the main pipeline)
    pc_d = nc.dram_tensor("pc", [PTILE, BLOC], f32, kind="ExternalInput")
    pd_d = nc.dram_tensor("pd", [PTILE, FH], f32, kind="ExternalInput")
    pseg_d = nc.dram_tensor("pseg", [BLOC, FH], f32, kind="ExternalOutput")

    NH_CHUNKS = 8
    CW = (TILES * T) // NH_CHUNKS

    with tile.TileContext(nc) as tc:
        with (
            tc.tile_pool(name="singles", bufs=1) as singles,
            tc.tile_pool(name="work", bufs=3) as work,
            tc.tile_pool(name="post", bufs=1) as post,
            tc.tile_pool(name="psacc", bufs=1, space="PSUM") as psacc,
            tc.tile_pool(name="psprobe", bufs=1, space="PSUM") as psprobe,
        ):
            NH = singles.tile([PTILE, TILES * T], f32)
            LINB = singles.tile([PTILE, F], f32)
            OHF = singles.tile([PTILE, TILES * BLOC], f32)
            for ch in range(NH_CHUNKS):
                nc.sync.dma_start(out=NH[:, ch * CW:(ch + 1) * CW],
                                  in_=nh_d.ap()[:, ch * CW:(ch + 1) * CW])
            nc.sync.dma_start(out=LINB, in_=linb_d.ap())
            nc.sync.dma_start(out=OHF, in_=oh_d.ap())
            OHR = singles.tile([PTILE, TILES * BLOC], f32r)
            nc.vector.tensor_copy(out=OHR, in_=OHF)

            linb3 = LINB.rearrange("p (r t) -> p r t", t=T)
            ect0 = psacc.tile([BLOC, F], f32, tag="ect0")
            ect1 = psacc.tile([BLOC, F], f32, tag="ect1")
            accs = (ect0, ect1)

            linb4 = LINB.rearrange("p (j r t) -> p j r t", j=1, t=T) \
                .broadcast_to([PTILE, 2, R, T])
            for g in range(GROUPS):
                z = work.tile([PTILE, GTILES * F], f32)
                if g % 2 == 0:
                    # variant A: one TT per pair of tiles (3 free dims)
                    for j in range(0, GTILES, 2):
                        i = GTILES * g + j
                        z4 = z[:, j * F:(j + 2) * F] \
                            .rearrange("p (j r t) -> p j r t", j=2, t=T)
                        nh4 = NH[:, i * T:(i + 2) * T] \
                            .rearrange("p (j r t) -> p j r t", j=2, r=1) \
                            .broadcast_to([PTILE, 2, R, T])
                        nc.vector.tensor_tensor(
                            out=z4, in0=linb4, in1=nh4,
                            op=mybir.AluOpType.subtract,
                        )
                else:
                    # variant B: one TT per tile (2 free dims)
                    for j in range(GTILES):
                        i = GTILES * g + j
                        z3 = z[:, j * F:(j + 1) * F] \
                            .rearrange("p (r t) -> p r t", t=T)
                        nh3 = NH[:, i * T:(i + 1) * T] \
                            .rearrange("p (r t) -> p r t", r=1) \
                            .broadcast_to([PTILE, R, T])
                        nc.vector.tensor_tensor(
                            out=z3, in0=linb3, in1=nh3,
                            op=mybir.AluOpType.subtract,
                        )

                ecc = work.tile([PTILE, GTILES * F], f32r)
                nc.scalar.activation(
                    out=ecc, in_=z,
                    func=mybir.ActivationFunctionType.Sigmoid,
                )

                for j in range(GTILES):
                    i = GTILES * g + j
                    acc = accs[i % 2]
                    for h in range(2):
                        nc.tensor.matmul(
                            out=acc[:, h * FH:(h + 1) * FH],
                            lhsT=OHR[:, i * BLOC:(i + 1) * BLOC],
                            rhs=ecc[:, j * F + h * FH:j * F + (h + 1) * FH],
                            start=(i < 2), stop=(i >= TILES - 2),
                        )

            # normalize: out = ect * (1 / max(ect, axis=free))
            ect1s = post.tile([BLOC, F], f32)
            nc.vector.tensor_copy(out=ect1s, in_=ect1)
            ectsum = post.tile([BLOC, F], f32)
            nc.vector.tensor_tensor(out=ectsum, in0=ect0, in1=ect1s,
                                    op=mybir.AluOpType.add)
            mx = post.tile([BLOC, 1], f32)
            nc.vector.tensor_reduce(
                out=mx, in_=ectsum,
                axis=mybir.AxisListType.X, op=mybir.AluOpType.max,
            )
            rmx = post.tile([BLOC, 1], f32)
            nc.vector.reciprocal(out=rmx, in_=mx)
            outn = post.tile([BLOC, F], f32)
            nc.vector.tensor_scalar(
                out=outn, in0=ectsum,
                scalar1=rmx, scalar2=None,
                op0=mybir.AluOpType.mult,
            )
            nc.sync.dma_start(out=out_d.ap(), in_=outn)

            # ---- fp32r probe ----
            PC = post.tile([PTILE, BLOC], f32)
            PD = post.tile([PTILE, FH], f32)
            nc.sync.dma_start(out=PC, in_=pc_d.ap())
            nc.sync.dma_start(out=PD, in_=pd_d.ap())
            PCr = post.tile([PTILE, BLOC], f32r)
            PDr = post.tile([PTILE, FH], f32r)
            nc.vector.tensor_copy(out=PCr, in_=PC)
            nc.vector.tensor_copy(out=PDr, in_=PD)
            pseg_ps = psprobe.tile([BLOC, FH], f32)
            nc.tensor.matmul(out=pseg_ps, lhsT=PCr,
                             rhs=PDr, start=True, stop=True)
            pseg_sb = post.tile([BLOC, FH], f32)
            nc.vector.tensor_copy(out=pseg_sb, in_=pseg_ps)
            nc.sync.dma_start(out=pseg_d.ap(), in_=pseg_sb)

    nc.compile()
    return nc


def _host_prep(x, v, lin, index):
    """Route points to their bin's core; build per-core input maps."""
    x = np.asarray(x, dtype=np.float32)
    v = np.asarray(v, dtype=np.float32)
    lin100 = (SCALE * np.asarray(lin, dtype=np.float32)).reshape(R)
    linb_row = np.repeat(lin100, T)                      # [F], f = r*T + t
    linb = np.ascontiguousarray(np.broadcast_to(linb_row, (PTILE, F)))

    order = np.argsort(index, kind="stable")
    counts = np.bincount(index, minlength=B)
    group_counts = counts.reshape(NCORES, BLOC).sum(axis=1)
    if group_counts.max() > CAP:
        return None  # fall back to host compute
    starts = np.concatenate([[0], np.cumsum(group_counts)[:-1]])

    nh100 = x @ (SCALE * v)                              # [N, T] f32

    # probe data (same for every core)
    rng = np.random.default_rng(0)
    pc = (rng.integers(0, BLOC, PTILE)[:, None]
          == np.arange(BLOC)[None, :]).astype(np.float32)
    pd = (1.0 / (1.0 + np.exp(-rng.standard_normal((PTILE, FH))))
          ).astype(np.float32)

    in_maps = []
    for c in range(NCORES):
        pts = order[starts[c]:starts[c] + group_counts[c]]
        n_c = len(pts)
        nh_c = np.zeros((CAP, T), dtype=np.float32)
        nh_c[:n_c] = nh100[pts]
        # nhT[p, i*T + t] = nh100 of point (i*PTILE + p)
        nhT = np.ascontiguousarray(
            nh_c.reshape(TILES, PTILE, T).transpose(1, 0, 2)
            .reshape(PTILE, TILES * T))
        idxf = np.full(CAP, -1, dtype=np.int64)
        idxf[:n_c] = index[pts] - c * BLOC
        # ohT[p, i*BLOC + b] = 1.0 iff point (i*PTILE + p) is in local bin b
        oh = (idxf.reshape(TILES, PTILE)[:, :, None]
              == np.arange(BLOC)[None, None, :]).astype(np.float32)
        ohT = np.ascontiguousarray(
            oh.transpose(1, 0, 2).reshape(PTILE, TILES * BLOC))
        in_maps.append({
            "nhT": nhT, "linb": linb, "ohT": ohT,
            "pc": pc, "pd": pd,
        })
    probes = {"pc": pc, "pd": pd}
    return in_maps, probes


def _host_fallback(x, v, lin, index):
    """Pure-numpy reference path (pathological index distributions only)."""
    x = np.asarray(x, dtype=np.float32)
    v = np.asarray(v, dtype=np.float32)
    lin = np.asarray(lin, dtype=np.float32).reshape(R, 1, 1)
    ect = np.zeros((B, R, T), dtype=np.float32)
    for s in range(0, len(x), 4096):
        xc = x[s:s + 4096]
        ic = index[s:s + 4096]
        nh = xc @ v                                   # [n, T]
        z = SCALE * (lin - nh[None, :, :])            # [R, n, T]
        ecc = 1.0 / (1.0 + np.exp(-z))
        np.add.at(ect, ic, np.transpose(ecc, (1, 0, 2)).astype(np.float32))
    return ect / ect.max(axis=(1, 2), keepdims=True)


def kernel(x, v, lin, index):
    from concourse import bass_utils

    x = np.asarray(x)
    v = np.asarray(v)
    lin = np.asarray(lin)
    index = np.asarray(index)

    prep = _host_prep(x, v, lin, index)
    if prep is None:
        return _host_fallback(x, v, lin, index)
    in_maps, _ = prep

    if "nc" not in _cache:
        _cache["nc"] = _build()
    nc = _cache["nc"]

    res = bass_utils.run_bass_kernel_spmd(nc, in_maps, list(range(NCORES)))
    out = np.concatenate(
        [res.results[c]["out"].reshape(BLOC, R, T) for c in range(NCORES)],
        axis=0,
    )
    return out.astype(np.float32)


# revision 24
# speedup vs baseline: 1.2710x; 1.0088x over previous
"""ECT layer (segment_reduce) Trainium2 kernel.

Math (matches the jax reference):
    nh  = x @ v                          [N, T]
    ecc = sigmoid(SCALE*(lin_r - nh))    [R, N, T]
    ect = segment_sum(ecc over N by index) -> [B, R, T]
    out = ect / max(ect over (R,T) per b)

Sharding: data-parallel over point clouds (bins). Core c owns global bins
[4c, 4c+4); the host routes every point to its bin's core, so no cross-core
reduction is needed. The host also precomputes the cheap [N,3]x[3,32]
projection nh (9.6 MFLOP) and the per-tile one-hot matrices; the device does
the heavy part (102M sigmoids + 102M-MAC segment reduction). Per core,
points are processed in 104 tiles of 128 (partition dim = points), fused in
groups of 4 tiles:
    DVE (3 tiles) + GPSIMD (1 tile): z = linb - nh (nh broadcast over R)
    ACT : ecc = sigmoid(z) over the whole group [128, 4096] (fp32r out)
    PE  : ect += onehot.T @ ecc  (fp32r, two alternating PSUM accumulators)
Epilogue: add the accumulators, per-bin max over R*T, multiply by
reciprocal, DMA out.
"""

import numpy as np

N = 100000
B = 32
R = 32
T = 32
D = 3
SCALE = 100.0

NCORES = 8
BLOC = B // NCORES        # local bins per core
CAP = 13312               # per-core point capacity (104 tiles of 128)
PTILE = 128
TILES = CAP // PTILE      # 104
GTILES = 8                # tiles per fused group
GROUPS = TILES // GTILES  # 13
TTILES = 4                # tiles per DVE tensor_tensor instruction
F = R * T                 # 1024 output features per bin
FH = F // 2               # 512, max moving free dim per matmul

_cache = {}


def _build():
    """Build + bacc-compile the SPMD program once per process."""
    import concourse.tile as tile
    from concourse import bacc, mybir

    nc = bacc.Bacc("TRN2", target_bir_lowering=False, debug=False,
                   num_devices=NCORES)
    f32 = mybir.dt.float32
    f32r = mybir.dt.float32r

    nh_d = nc.dram_tensor("nhT", [PTILE, TILES * T], f32,
                          kind="ExternalInput")
    linb_d = nc.dram_tensor("linb", [PTILE, F], f32, kind="ExternalInput")
    oh_d = nc.dram_tensor("ohT", [PTILE, TILES * BLOC], f32,
                          kind="ExternalInput")
    out_d = nc.dram_tensor("out", [BLOC, F], f32, kind="ExternalOutput")

    # fp32r accuracy probe (runs once, independent of the main pipeline)
    pc_d = nc.dram_tensor("pc", [PTILE, BLOC], f32, kind="ExternalInput")
    pd_d = nc.dram_tensor("pd", [PTILE, FH], f32, kind="ExternalInput")
    pseg_d = nc.dram_tensor("pseg", [BLOC, FH], f32, kind="ExternalOutput")

    NH_CHUNKS = 8
    CW = (TILES * T) // NH_CHUNKS

    with tile.TileContext(nc) as tc:
        with (
            tc.tile_pool(name="singles", bufs=1) as singles,
            tc.tile_pool(name="work", bufs=2) as work,
            tc.tile_pool(name="post", bufs=1) as post,
            tc.tile_pool(name="psacc", bufs=1, space="PSUM") as psacc,
            tc.tile_pool(name="psprobe", bufs=1, space="PSUM") as psprobe,
        ):
            NH = singles.tile([PTILE, TILES * T], f32)
            LINB = singles.tile([PTILE, F], f32)
            OHF = singles.tile([PTILE, TILES * BLOC], f32)
            nc.sync.dma_start(out=LINB, in_=linb_d.ap())
            nc.sync.dma_start(out=OHF, in_=oh_d.ap())
            for ch in range(NH_CHUNKS):
                nc.sync.dma_start(out=NH[:, ch * CW:(ch + 1) * CW],
                                  in_=nh_d.ap()[:, ch * CW:(ch + 1) * CW])
            OHR = singles.tile([PTILE, TILES * BLOC], f32r)
            nc.vector.tensor_copy(out=OHR, in_=OHF)

            # ---- fp32r probe (scheduled early; PE is idle at startup) ----
            PC = post.tile([PTILE, BLOC], f32)
            PD = post.tile([PTILE, FH], f32)
            nc.sync.dma_start(out=PC, in_=pc_d.ap())
            nc.sync.dma_start(out=PD, in_=pd_d.ap())
            PCr = post.tile([PTILE, BLOC], f32r)
            PDr = post.tile([PTILE, FH], f32r)
            nc.vector.tensor_copy(out=PCr, in_=PC)
            nc.vector.tensor_copy(out=PDr, in_=PD)
            pseg_ps = psprobe.tile([BLOC, FH], f32)
            nc.tensor.matmul(out=pseg_ps, lhsT=PCr,
                             rhs=PDr, start=True, stop=True)
            pseg_sb = post.tile([BLOC, FH], f32)
            nc.vector.tensor_copy(out=pseg_sb, in_=pseg_ps)
            nc.sync.dma_start(out=pseg_d.ap(), in_=pseg_sb)

            linb3 = LINB.rearrange("p (r t) -> p r t", t=T)
            ect0 = psacc.tile([BLOC, F], f32, tag="ect0")
            ect1 = psacc.tile([BLOC, F], f32, tag="ect1")
            accs = (ect0, ect1)

            linbT = LINB.rearrange("p (j r t) -> p j r t", j=1, t=T) \
                .broadcast_to([PTILE, TTILES, R, T])
            for g in range(GROUPS):
                z = work.tile([PTILE, GTILES * F], f32)
                for j in range(0, GTILES, TTILES):
                    i = GTILES * g + j
                    z4 = z[:, j * F:(j + TTILES) * F] \
                        .rearrange("p (j r t) -> p j r t", j=TTILES, t=T)
                    nh4 = NH[:, i * T:(i + TTILES) * T] \
                        .rearrange("p (j r t) -> p j r t", j=TTILES, r=1) \
                        .broadcast_to([PTILE, TTILES, R, T])
                    nc.vector.tensor_tensor(
                        out=z4, in0=linbT, in1=nh4,
                        op=mybir.AluOpType.subtract,
                    )

                ecc = work.tile([PTILE, GTILES * F], f32r)
                nc.scalar.activation(
                    out=ecc, in_=z,
                    func=mybir.ActivationFunctionType.Sigmoid,
                )

                for j in range(GTILES):
                    i = GTILES * g + j
                    acc = accs[i % 2]
                    for h in range(2):
                        nc.tensor.matmul(
                            out=acc[:, h * FH:(h + 1) * FH],
                            lhsT=OHR[:, i * BLOC:(i + 1) * BLOC],
                            rhs=ecc[:, j * F + h * FH:j * F + (h + 1) * FH],
                            start=(i < 2), stop=(i >= TILES - 2),
                        )

            # normalize: out = ect * (1 / max(ect, axis=free))
            ect1s = post.tile([BLOC, F], f32)
            nc.scalar.copy(out=ect1s, in_=ect1)
            ectsum = post.tile([BLOC, F], f32)
            nc.vector.tensor_tensor(out=ectsum, in0=ect0, in1=ect1s,
                                    op=mybir.AluOpType.add)
            mx = post.tile([BLOC, 1], f32)
            nc.vector.tensor_reduce(
                out=mx, in_=ectsum,
                axis=mybir.AxisListType.X, op=mybir.AluOpType.max,
            )
            rmx = post.tile([BLOC, 1], f32)
            nc.vector.reciprocal(out=rmx, in_=mx)
            outn = post.tile([BLOC, F], f32)
            nc.vector.tensor_scalar(
                out=outn, in0=ectsum,
                scalar1=rmx, scalar2=None,
                op0=mybir.AluOpType.mult,
            )
            nc.sync.dma_start(out=out_d.ap(), in_=outn)

    nc.compile()
    return nc


def _host_prep(x, v, lin, index):
    """Route points to their bin's core; build per-core input maps."""
    x = np.asarray(x, dtype=np.float32)
    v = np.asarray(v, dtype=np.float32)
    lin100 = (SCALE * np.asarray(lin, dtype=np.float32)).reshape(R)
    linb_row = np.repeat(lin100, T)                      # [F], f = r*T + t
    linb = np.ascontiguousarray(np.broadcast_to(linb_row, (PTILE, F)))

    order = np.argsort(index, kind="stable")
    counts = np.bincount(index, minlength=B)
    group_counts = counts.reshape(NCORES, BLOC).sum(axis=1)
    if group_counts.max() > CAP:
        return None  # fall back to host compute
    starts = np.concatenate([[0], np.cumsum(group_counts)[:-1]])

    nh100 = x @ (SCALE * v)                              # [N, T] f32

    # probe data (same for every core)
    rng = np.random.default_rng(0)
    pc = (rng.integers(0, BLOC, PTILE)[:, None]
          == np.arange(BLOC)[None, :]).astype(np.float32)
    pd = (1.0 / (1.0 + np.exp(-rng.standard_normal((PTILE, FH))))
          ).astype(np.float32)

    in_maps = []
    for c in range(NCORES):
        pts = order[starts[c]:starts[c] + group_counts[c]]
        n_c = len(pts)
        nh_c = np.zeros((CAP, T), dtype=np.float32)
        nh_c[:n_c] = nh100[pts]
        # nhT[p, i*T + t] = nh100 of point (i*PTILE + p)
        nhT = np.ascontiguousarray(
            nh_c.reshape(TILES, PTILE, T).transpose(1, 0, 2)
            .reshape(PTILE, TILES * T))
        idxf = np.full(CAP, -1, dtype=np.int64)
        idxf[:n_c] = index[pts] - c * BLOC
        # ohT[p, i*BLOC + b] = 1.0 iff point (i*PTILE + p) is in local bin b
        oh = (idxf.reshape(TILES, PTILE)[:, :, None]
              == np.arange(BLOC)[None, None, :]).astype(np.float32)
        ohT = np.ascontiguousarray(
            oh.transpose(1, 0, 2).reshape(PTILE, TILES * BLOC))
        in_maps.append({
            "nhT": nhT, "linb": linb, "ohT": ohT,
            "pc": pc, "pd": pd,
        })
    probes = {"pc": pc, "pd": pd}
    return in_maps, probes


def _host_fallback(x, v, lin, index):
    """Pure-numpy reference path (pathological index distributions only)."""
    x = np.asarray(x, dtype=np.float32)
    v = np.asarray(v, dtype=np.float32)
    lin = np.asarray(lin, dtype=np.float32).reshape(R, 1, 1)
    ect = np.zeros((B, R, T), dtype=np.float32)
    for s in range(0, len(x), 4096):
        xc = x[s:s + 4096]
        ic = index[s:s + 4096]
        nh = xc @ v                                   # [n, T]
        z = SCALE * (lin - nh[None, :, :])            # [R, n, T]
        ecc = 1.0 / (1.0 + np.exp(-z))
        np.add.at(ect, ic, np.transpose(ecc, (1, 0, 2)).astype(np.float32))
    return ect / ect.max(axis=(1, 2), keepdims=True)


def kernel(x, v, lin, index):
    from concourse import bass_utils

    x = np.asarray(x)
    v = np.asarray(v)
    lin = np.asarray(lin)
    index = np.asarray(index)

    prep = _host_prep(x, v, lin, index)
    if prep is None:
        return _host_fallback(x, v, lin, index)
    in_maps, _ = prep

    if "nc" not in _cache:
        _cache["nc"] = _build()
    nc = _cache["nc"]

    res = bass_utils.run_bass_kernel_spmd(nc, in_maps, list(range(NCORES)))
    out = np.concatenate(
        [res.results[c]["out"].reshape(BLOC, R, T) for c in range(NCORES)],
        axis=0,
    )
    return out.astype(np.float32)
